# revision 1
# baseline (speedup 1.0000x reference)
"""Trainium2 Bass kernel for nn_AttentionHead (single-head attention with
pre-softmax tril zeroing). B=8, S=2048, E=1024, H=64.

Sharding: data-parallel over batch - one batch element per NeuronCore,
no collectives. Each core computes, for its batch b:

  q = y@Wq + bq ; k' = x@(Wk/8) + (bk/8) ; v = x@Wv + bv
  scores[r, j] = q[r].k'[j] for j<=r, 0 for j>r      (tril PRE-softmax)
  attn = softmax(scores, -1)  -> masked entries contribute exp(0)=1
  out = attn @ v

v10 design - slab pipeline:
  - x,y host-cast to bf16, host-pre-transposed to [E, S], and laid out as
    4 s-range SLABS of shape [128, ECH*512] (partition-major) so qkv
    chunk i and attention column i unblock as soon as slab i lands
  - per slab: 16 projection matmuls (kv+q interleaved across 2 PSUM
    accumulators), evac kT(ACT)/qT(DVE)/vT(ACT or DVE), SBUF-SBUF DMA
    duplicates of kT/qT onto partitions 64:128, xbar DMA-transpose of
    the vT slab into natural layout with a fused ones-column
  - attention column c follows slab c: scores transposed (st[k, q]) in
    row-packed PAIRS via tile_position (0,0)/(64,0); exp on ACT into
    bf16; diagonal blocks exp only the live range and GpSimd fills
    masked cells with exp(0)=1; pv accumulates [v|1]^T @ ex in PSUM
  - each column's pv is copied to SBUF right after its last block so the
    bank recycles; the closed-form upper-triangle add (suffix sums of v,
    which need the last slab) + normalize + store run as a tail
  - two fp32 dummy matmuls at t=0 warm the PE HAM clock gate
"""

import numpy as np

import concourse.bass as bass
import concourse.mybir as mybir
from concourse.tile import TileContext

S, E, H = 2048, 1024, 64
SC = S // 128   # 16 s-chunks (key blocks)
ECH = E // 128  # 8 e-chunks
NQ = 4          # q-chunks (slabs) of 512
F32 = mybir.dt.float32
BF16 = mybir.dt.bfloat16
AF = mybir.ActivationFunctionType

_SPLIT_COUNTER = [0]


def _split_multi_waits(nc, ev_cap=1):
    """This container's walrus build accepts at most 1 sem-wait per
    instruction (2 on EventSemaphore); move excess waits onto EvSem
    instructions inserted just before, on the same engine."""
    for f in nc.m.functions:
        for bb in f.blocks:
            ins_list = bb.instructions
            need = False
            for ins in ins_list:
                si = ins.sync_info
                if si is None:
                    continue
                cap = 2 if isinstance(ins, mybir.InstEventSemaphore) else 1
                if len(si.on_wait) > cap:
                    need = True
                    break
            if not need:
                continue
            new_list = []
            for ins in ins_list:
                si = ins.sync_info
                cap = 2 if isinstance(ins, mybir.InstEventSemaphore) else 1
                if si is not None and len(si.on_wait) > cap:
                    waits = list(si.on_wait)
                    keep = waits[-cap:]
                    head = waits[:-cap]
                    for i in range(0, len(head), ev_cap):
                        _SPLIT_COUNTER[0] += 1
                        ev = mybir.InstEventSemaphore(
                            name=f"EVSPLIT-{_SPLIT_COUNTER[0]}",
                            engine=ins.engine,
                            ins=[],
                            outs=[],
                            sync_info=mybir.SyncInfo(
                                on_wait=head[i:i + ev_cap], on_update=[]
                            ),
                        )
                        nc.register_instruction(ev)
                        new_list.append(ev)
                    ins.sync_info = mybir.SyncInfo(
                        on_wait=keep, on_update=list(si.on_update)
                    )
                new_list.append(ins)
            bb.instructions = new_list


def _build():
    nc = bass.Bass()
    # x, y: [NQ slabs, 128, ECH*512] bf16 (host-packed, see kernel())
    x_ext = nc.declare_dram_parameter("x", [NQ, 128, ECH * 512], BF16,
                                      isOutput=False)
    y_ext = nc.declare_dram_parameter("y", [NQ, 128, ECH * 512], BF16,
                                      isOutput=False)
    wkv_ext = nc.declare_dram_parameter("wkv", [128, ECH * 128], BF16,
                                        isOutput=False)
    wq_ext = nc.declare_dram_parameter("wq", [128, ECH * H], BF16,
                                       isOutput=False)
    bq_ext = nc.declare_dram_parameter("bq", [H, 1], F32, isOutput=False)
    bk_ext = nc.declare_dram_parameter("bk", [H, 1], F32, isOutput=False)
    bv_ext = nc.declare_dram_parameter("bv", [H, 1], F32, isOutput=False)
    out_ext = nc.declare_dram_parameter("out", [S, H], F32, isOutput=True)

    with TileContext(nc) as tc:
        with (
            tc.tile_pool(name="consts", bufs=1) as consts,
            tc.tile_pool(name="bigT", bufs=1) as bigT,
            tc.tile_pool(name="expp", bufs=3) as expp,
            tc.tile_pool(name="outp", bufs=2) as outp,
        ):
            # ---- constants ----
            ident_f = consts.tile([128, 128], F32)
            nc.vector.memset(ident_f, 1.0)
            nc.gpsimd.affine_select(
                out=ident_f, in_=ident_f,
                pattern=[[-1, 128]], channel_multiplier=1, base=0,
                compare_op=mybir.AluOpType.is_equal, fill=0.0,
            )
            ident_bf = consts.tile([128, 128], BF16)
            nc.vector.memset(ident_bf, 1.0)
            nc.gpsimd.affine_select(
                out=ident_bf, in_=ident_bf,
                pattern=[[-1, 128]], channel_multiplier=1, base=0,
                compare_op=mybir.AluOpType.is_equal, fill=0.0,
            )

            # ---- PE warm-up: two fp32 matmuls (~3.4us) flip the HAM ----
            scr = consts.tile([128, 512], F32, tag="scr")
            nc.vector.memset(scr, 0.0)
            with tc.tile_pool(name="psW", bufs=1, space="PSUM") as psW:
                wm = psW.tile([128, 512], F32, tag="warm")
                for _ in range(2):
                    nc.tensor.matmul(wm, lhsT=ident_f, rhs=scr,
                                     start=True, stop=True)

            # ---- input slabs + weights; biases on SWDGE ----
            w_kv = consts.tile([128, ECH * 128], BF16, tag="w_kv")
            w_q = consts.tile([128, ECH * H], BF16, tag="w_q")
            bias_sb = {}
            for name, bext in (("q", bq_ext), ("k", bk_ext), ("v", bv_ext)):
                bs = consts.tile([H, 1], F32, tag=f"b_{name}",
                                 name=f"bias_{name}")
                nc.gpsimd.dma_start(out=bs, in_=bext[:, :])
                bias_sb[name] = bs

            xTs = [bigT.tile([128, ECH * 512], BF16, tag=f"xT{i}",
                             name=f"xT_{i}") for i in range(NQ)]
            yTs = [bigT.tile([128, ECH * 512], BF16, tag=f"yT{i}",
                             name=f"yT_{i}") for i in range(NQ)]
            # ALL slab loads on the sync queue (both HWDGE queues share
            # the same 16 SDMA rings, so one queue gets full bandwidth);
            # the ACT queue stays free for exps from the start
            nc.sync.dma_start(out=xTs[0], in_=x_ext[0])
            nc.sync.dma_start(out=w_kv, in_=wkv_ext[:, :])
            nc.scalar.dma_start(out=w_q, in_=wq_ext[:, :])
            nc.sync.dma_start(out=yTs[0], in_=y_ext[0])
            for i in range(1, NQ):
                nc.sync.dma_start(out=xTs[i], in_=x_ext[i])
                nc.sync.dma_start(out=yTs[i], in_=y_ext[i])

            # kT/qT duplicated on partitions 64:128 for row-packed scores
            qTd = bigT.tile([128, S], BF16, tag="qTd")
            kTd = bigT.tile([128, S], BF16, tag="kTd")
            vT = bigT.tile([H, S], BF16, tag="vT")
            v_aug = bigT.tile([128, SC * (H + 1)], BF16, tag="vaug")
            nc.gpsimd.memset(v_aug, 1.0)
            v_nat = bigT.tile([128, SC * H], BF16, tag="vnat")
            vsuf = []
            wins = [None]
            for c in range(NQ):
                va = consts.tile([H + 1, 1], F32, tag=f"vsuf{c}",
                                 name=f"vsuf_{c}")
                nc.vector.memset(va, 0.0)
                if c < NQ - 1:
                    nc.vector.memset(va[H:H + 1, :],
                                     float((NQ - 1 - c) * 512))
                vsuf.append(va)
            for g in (1, 2):
                wins.append(consts.tile([H, 1], F32, tag=f"win{g}",
                                        name=f"win_{g}"))

            sbns = []
            with (
                tc.tile_pool(name="psQ", bufs=1, space="PSUM") as psQ,
                tc.tile_pool(name="psE", bufs=1, space="PSUM") as psE,
            ):
                def finish(c):
                    # closed-form upper add + normalize + store col c
                    sbn = sbns[c]
                    if c < NQ - 1:
                        nc.vector.tensor_scalar_add(out=sbn, in0=sbn,
                                                    scalar1=vsuf[c])
                    pt4 = psE.tile([128, 4 * (H + 4)], BF16, tag="pt",
                                   bufs=1, name=f"pt4_{c}")
                    pt4v = pt4.rearrange("p (j h) -> p j h", h=H + 4)
                    for j4 in range(4):
                        nc.tensor.transpose(
                            pt4[:, j4 * (H + 4):j4 * (H + 4) + H + 1],
                            sbn[:, j4 * 128:(j4 + 1) * 128],
                            ident_bf[0:H + 1, 0:H + 1],
                        )
                    rcp4 = outp.tile([128, 4], F32, tag="rcp",
                                     name=f"rcp4_{c}")
                    nc.vector.reciprocal(
                        rcp4.rearrange("p (j o) -> p j o", o=1),
                        pt4v[:, :, H:H + 1])
                    of4 = outp.tile([128, 4 * H], F32, tag="of",
                                    name=f"of4_{c}")
                    of4v = of4.rearrange("p (j h) -> p j h", h=H)
                    for j4 in range(4):
                        nc.vector.tensor_scalar_mul(
                            out=of4v[:, j4, :], in0=pt4v[:, j4, 0:H],
                            scalar1=rcp4[:, j4:j4 + 1])
                    nc.sync.dma_start(
                        out=out_ext[c * 512:(c + 1) * 512, :].rearrange(
                            "(j p) h -> p j h", p=128),
                        in_=of4v)

                for i in range(NQ):
                    # ---- projections for slab i ----
                    kv_acc = psQ.tile([128, 512], F32, tag="kvacc", bufs=1,
                                      name=f"kvacc_{i}")
                    q_acc = psQ.tile([H, 512], F32, tag="qacc", bufs=1,
                                     name=f"qacc_{i}")
                    # slab 0: kv first (x lands before y), else interleave
                    order = ([("kv", e) for e in range(ECH)]
                             + [("q", e) for e in range(ECH)]) if i == 0 else \
                        [t for e in range(ECH) for t in (("kv", e), ("q", e))]
                    for kind, e in order:
                        if kind == "kv":
                            nc.tensor.matmul(
                                kv_acc,
                                lhsT=w_kv[:, e * 128:(e + 1) * 128],
                                rhs=xTs[i][:, e * 512:(e + 1) * 512],
                                start=(e == 0),
                                stop=(e == ECH - 1),
                            )
                        else:
                            nc.tensor.matmul(
                                q_acc,
                                lhsT=w_q[:, e * H:(e + 1) * H],
                                rhs=yTs[i][:, e * 512:(e + 1) * 512],
                                start=(e == 0),
                                stop=(e == ECH - 1),
                            )
                    sl = slice(i * 512, (i + 1) * 512)
                    nc.vector.tensor_scalar_add(
                        out=kTd[0:H, sl], in0=kv_acc[0:H, :],
                        scalar1=bias_sb["k"])
                    nc.vector.tensor_scalar_add(
                        out=qTd[0:H, sl], in0=q_acc, scalar1=bias_sb["q"])
                    nc.scalar.dma_start(out=kTd[H:128, sl],
                                        in_=kTd[0:H, sl])
                    nc.scalar.dma_start(out=qTd[H:128, sl],
                                        in_=qTd[0:H, sl])
                    nc.vector.tensor_scalar_add(
                        out=vT[:, sl], in0=kv_acc[H:128, :],
                        scalar1=bias_sb["v"])
                    nc.scalar.dma_start(
                        out=v_nat.rearrange(
                            "p (j h) -> p j h", h=H
                        )[:, 4 * i:4 * i + 4, :],
                        in_=vT[:, sl], transpose=True,
                    )
                    nc.gpsimd.tensor_copy(
                        v_aug.rearrange(
                            "p (j h) -> p j h", h=H + 1
                        )[:, 4 * i:4 * i + 4, 0:H],
                        v_nat.rearrange(
                            "p (j h) -> p j h", h=H
                        )[:, 4 * i:4 * i + 4, :],
                    )
                    if i in (1, 2):
                        nc.vector.reduce_sum(
                            out=wins[i], in_=vT[:, sl],
                            axis=mybir.AxisListType.X)
                    elif i == 3:
                        nc.vector.reduce_sum(
                            out=vsuf[2][0:H, :], in_=vT[:, sl],
                            axis=mybir.AxisListType.X)
                        nc.vector.tensor_add(
                            out=vsuf[1][0:H, :], in0=vsuf[2][0:H, :],
                            in1=wins[2])
                        nc.vector.tensor_add(
                            out=vsuf[0][0:H, :], in0=vsuf[1][0:H, :],
                            in1=wins[1])

                    # ---- attention column i ----
                    c = i
                    pv = psE.tile([H + 1, 512], F32, tag="pv", bufs=1,
                                  name=f"pv_{c}")
                    nb = 4 * c + 4
                    for b2 in range(nb // 2):
                        # one two-bank tile per score pair -> a single
                        # fused 1024-col exp for non-diagonal pairs
                        st2 = psE.tile([128, 1024], F32, tag="st",
                                       bufs=2, name=f"st_{c}_{b2}")
                        ex2 = expp.tile([128, 1024], BF16, tag="expst",
                                        bufs=20, name=f"ex_{c}_{b2}")
                        diag = (2 * b2) // 4 == c
                        for half in range(2):
                            b = 2 * b2 + half
                            lo = half * H
                            o = half * 512
                            d0 = 128 * (b - 4 * c) if diag else 0
                            nc.tensor.matmul(
                                st2[:, o + d0:o + 512],
                                lhsT=kTd[lo:lo + H, b * 128:(b + 1) * 128],
                                rhs=qTd[lo:lo + H,
                                        c * 512 + d0:(c + 1) * 512],
                                start=True,
                                stop=True,
                                tile_position=(lo, 0),
                            )
                        if diag:
                            for half in range(2):
                                b = 2 * b2 + half
                                d = b - 4 * c
                                o = half * 512
                                nc.scalar.activation(
                                    out=ex2[:, o + 128 * d:o + 512],
                                    in_=st2[:, o + 128 * d:o + 512],
                                    func=AF.Exp)
                                w = 128 * (d + 1)
                                nc.gpsimd.affine_select(
                                    out=ex2[:, o:o + w],
                                    in_=ex2[:, o:o + w],
                                    pattern=[[1, w]], channel_multiplier=-1,
                                    base=-128 * d,
                                    compare_op=mybir.AluOpType.is_ge,
                                    fill=1.0,
                                )
                        else:
                            nc.scalar.activation(out=ex2, in_=st2,
                                                 func=AF.Exp)
                        for half in range(2):
                            b = 2 * b2 + half
                            nc.tensor.matmul(
                                pv,
                                lhsT=v_aug[:, b * (H + 1):(b + 1) * (H + 1)],
                                rhs=ex2[:, half * 512:(half + 1) * 512],
                                start=(b == 0),
                                stop=(b == nb - 1),
                            )
                    # evacuate pv -> SBUF immediately (bank recycles);
                    # closed-form add + normalize happen in the tail
                    sbn = outp.tile([H + 1, 512], BF16, tag="sbn", bufs=4,
                                    name=f"sbn_{c}")
                    nc.vector.tensor_copy(sbn, pv)
                    sbns.append(sbn)

                # ---- finishes for cols 2,3 (0,1 were emitted after
                # col 2 so their DVE/DMA chains overlap col 3) ----
                for c in range(NQ):
                    finish(c)

    _split_multi_waits(nc)
    return nc


LAST_EXEC_TIME_NS = None
_CACHE = {}


def kernel(x, y, Wq, bq, Wk, bk, Wv, bv):
    """Full-input entry point: shards batch over 8 NeuronCores (one batch
    element per core), runs the Bass kernel, gathers the full output."""
    global LAST_EXEC_TIME_NS
    import os

    import ml_dtypes
    from concourse.bass_utils import run_bass_kernel_spmd

    if "nc" not in _CACHE:
        _CACHE["nc"] = _build()
    nc = _CACHE["nc"]

    bf = ml_dtypes.bfloat16
    x = np.asarray(x, np.float32)
    y = np.asarray(y, np.float32)

    # host-side weight packing: [128, ECH, 128] -> [128, ECH*128]
    wk8 = (np.asarray(Wk, np.float32) * 0.125).astype(bf).reshape(ECH, 128, H)
    wv2 = np.asarray(Wv, np.float32).astype(bf).reshape(ECH, 128, H)
    wkv = np.ascontiguousarray(
        np.concatenate([wk8, wv2], axis=2).transpose(1, 0, 2)
    ).reshape(128, ECH * 128)
    wq2 = np.ascontiguousarray(
        np.asarray(Wq, np.float32).astype(bf).reshape(ECH, 128, H)
        .transpose(1, 0, 2)
    ).reshape(128, ECH * H)
    bqc = np.ascontiguousarray(np.asarray(bq, np.float32).reshape(H, 1))
    bkc = np.ascontiguousarray(
        np.asarray(bk, np.float32).reshape(H, 1) * 0.125)
    bvc = np.ascontiguousarray(np.asarray(bv, np.float32).reshape(H, 1))

    def slabs(a):
        # [S, E] f32 -> bf16 [E, S] -> [NQ, 128, ECH*512] slab-major:
        # slab i, partition p, (e, s) -> a.T[e*128+p, i*512+s]
        t = a.astype(bf).T.reshape(ECH, 128, NQ, 512)
        return np.ascontiguousarray(
            t.transpose(2, 1, 0, 3).reshape(NQ, 128, ECH * 512))

    in_maps = []
    for b in range(8):
        in_maps.append({
            "x": slabs(x[b]), "y": slabs(y[b]),
            "wkv": wkv, "wq": wq2,
            "bq": bqc, "bk": bkc, "bv": bvc,
        })

    trace = bool(os.environ.get("ATTN_TRACE"))
    res = run_bass_kernel_spmd(nc, in_maps, core_ids=list(range(8)),
                               trace=trace)
    if trace:
        LAST_EXEC_TIME_NS = res.exec_time_ns
    return np.stack([res.results[i]["out"] for i in range(8)]).astype(
        np.float32)



# revision 5
# speedup vs baseline: 1.0265x; 1.0265x over previous
"""Trainium2 Bass kernel for nn_AttentionHead (single-head attention with
pre-softmax tril zeroing). B=8, S=2048, E=1024, H=64.

Sharding: data-parallel over batch - one batch element per NeuronCore.

v11 design (from v10 trace analysis):
  - input DMAs all on the sync HWDGE queue, emitted first, in order
    wkv, x0, wqq, y0, y1, x1, y2, y3, x2, x3; kT dups moved to Pool
    SWDGE and v-transposes to the Act queue so the input stream never
    serializes behind compute-dependent DMAs (v10's 14us DMA hole)
  - bf16 warmup matmul chain keeps the PE busy (and its p-state ramped)
    through the ~11us DMA/program preamble; it shares the kv_acc PSUM
    bank so slab 0's first matmul chains with zero gap
  - q projection uses [Wq|Wq] (M=128): partitions 64:128 of qTd are
    produced by the matmul itself - no SBUF-SBUF dup DMA
  - diagonal pv blocks are d0-trimmed like the scores; the skipped
    all-ones columns are replaced by closed-form per-128-step
    corrections applied POST-transpose: denominator constants are
    compile-time memsets, v-suffix sums come from ones[128,128] @ v_nat
    matmuls (partition-replicated for free). Each column's transpose/
    reciprocal runs right after its pv; only a [128,256] add + 4 muls +
    store remain for the tail -> tail shrinks from ~12us to ~2us
"""

import numpy as np

import concourse.bass as bass
import concourse.mybir as mybir
from concourse.tile import TileContext

S, E, H = 2048, 1024, 64
SC = S // 128   # 16 key blocks
ECH = E // 128  # 8 e-chunks
NQ = 4          # q-chunks (slabs) of 512
F32 = mybir.dt.float32
BF16 = mybir.dt.bfloat16
AF = mybir.ActivationFunctionType
N_WARM = 26     # bf16 warmup matmuls covering the DMA/program preamble

_SPLIT_COUNTER = [0]


def _split_multi_waits(nc, ev_cap=1):
    """This container's walrus build accepts at most 1 sem-wait per
    instruction (2 on EventSemaphore); move excess waits onto EvSem
    instructions inserted just before, on the same engine."""
    for f in nc.m.functions:
        for bb in f.blocks:
            ins_list = bb.instructions
            need = False
            for ins in ins_list:
                si = ins.sync_info
                if si is None:
                    continue
                cap = 2 if isinstance(ins, mybir.InstEventSemaphore) else 1
                if len(si.on_wait) > cap:
                    need = True
                    break
            if not need:
                continue
            new_list = []
            for ins in ins_list:
                si = ins.sync_info
                cap = 2 if isinstance(ins, mybir.InstEventSemaphore) else 1
                if si is not None and len(si.on_wait) > cap:
                    waits = list(si.on_wait)
                    keep = waits[-cap:]
                    head = waits[:-cap]
                    for i in range(0, len(head), ev_cap):
                        _SPLIT_COUNTER[0] += 1
                        ev = mybir.InstEventSemaphore(
                            name=f"EVSPLIT-{_SPLIT_COUNTER[0]}",
                            engine=ins.engine,
                            ins=[],
                            outs=[],
                            sync_info=mybir.SyncInfo(
                                on_wait=head[i:i + ev_cap], on_update=[]
                            ),
                        )
                        nc.register_instruction(ev)
                        new_list.append(ev)
                    ins.sync_info = mybir.SyncInfo(
                        on_wait=keep, on_update=list(si.on_update)
                    )
                new_list.append(ins)
            bb.instructions = new_list


def _build():
    nc = bass.Bass()
    x_ext = nc.declare_dram_parameter("x", [NQ, 128, ECH * 512], BF16,
                                      isOutput=False)
    y_ext = nc.declare_dram_parameter("y", [NQ, 128, ECH * 512], BF16,
                                      isOutput=False)
    wkv_ext = nc.declare_dram_parameter("wkv", [128, ECH * 128], BF16,
                                        isOutput=False)
    wqq_ext = nc.declare_dram_parameter("wqq", [128, ECH * 128], BF16,
                                        isOutput=False)
    bqq_ext = nc.declare_dram_parameter("bqq", [128, 1], F32, isOutput=False)
    bkv_ext = nc.declare_dram_parameter("bkv", [128, 1], F32, isOutput=False)
    out_ext = nc.declare_dram_parameter("out", [S, H], F32, isOutput=True)

    with TileContext(nc) as tc:
        with (
            tc.tile_pool(name="consts", bufs=1) as consts,
            tc.tile_pool(name="bigT", bufs=1) as bigT,
            tc.tile_pool(name="expp", bufs=3) as expp,
            tc.tile_pool(name="outp", bufs=2) as outp,
        ):
            # ---- constants ----
            ident_bf = consts.tile([128, 128], BF16)
            nc.vector.memset(ident_bf, 1.0)
            nc.gpsimd.affine_select(
                out=ident_bf, in_=ident_bf,
                pattern=[[-1, 128]], channel_multiplier=1, base=0,
                compare_op=mybir.AluOpType.is_equal, fill=0.0,
            )
            ones_bf = consts.tile([128, 128], BF16, tag="ones")
            nc.gpsimd.memset(ones_bf, 1.0)
            # warmup source (content irrelevant; memset for sim cleanliness)
            scr = consts.tile([128, 512], BF16, tag="scr")
            nc.vector.memset(scr, 0.0)
            # denominator constants per (col c, step d): S-128-512c-128d
            denC = consts.tile([128, 16], F32, tag="denC")
            for c in range(NQ):
                for dstep in range(4):
                    nc.vector.memset(
                        denC[:, 4 * c + dstep:4 * c + dstep + 1],
                        float(S - 128 - 512 * c - 128 * dstep))

            # ---- biases on SWDGE ----
            bqq = consts.tile([128, 1], F32, tag="bqq", name="bias_qq")
            nc.gpsimd.dma_start(out=bqq, in_=bqq_ext[:, :])
            bkv = consts.tile([128, 1], F32, tag="bkv", name="bias_kv")
            nc.gpsimd.dma_start(out=bkv, in_=bkv_ext[:, :])

            # ---- weights + input slabs: sync queue, load order matters ----
            w_kv = consts.tile([128, ECH * 128], BF16, tag="w_kv")
            w_qq = consts.tile([128, ECH * 128], BF16, tag="w_qq")
            xTs = [bigT.tile([128, ECH * 512], BF16, tag=f"xT{i}",
                             name=f"xT_{i}") for i in range(NQ)]
            yTs = [bigT.tile([128, ECH * 512], BF16, tag=f"yT{i}",
                             name=f"yT_{i}") for i in range(NQ)]
            nc.sync.dma_start(out=w_kv, in_=wkv_ext[:, :])
            nc.sync.dma_start(out=xTs[0], in_=x_ext[0])
            nc.sync.dma_start(out=w_qq, in_=wqq_ext[:, :])
            nc.sync.dma_start(out=yTs[0], in_=y_ext[0])
            nc.sync.dma_start(out=yTs[1], in_=y_ext[1])
            nc.sync.dma_start(out=xTs[1], in_=x_ext[1])
            nc.sync.dma_start(out=yTs[2], in_=y_ext[2])
            nc.sync.dma_start(out=yTs[3], in_=y_ext[3])
            nc.sync.dma_start(out=xTs[2], in_=x_ext[2])
            nc.sync.dma_start(out=xTs[3], in_=x_ext[3])

            # qTd: both halves from [Wq|Wq]; kTd: rows 0:64 evac, 64:128 dup
            qTd = bigT.tile([128, S], BF16, tag="qTd")
            kTd = bigT.tile([128, S], BF16, tag="kTd")
            vT = bigT.tile([64, S], BF16, tag="vT")
            v_nat = bigT.tile([128, SC * H], BF16, tag="vnat")
            v_aug = bigT.tile([128, SC * (H + 1)], BF16, tag="vaug")
            nc.gpsimd.memset(v_aug, 1.0)
            # per-slab block-sum (replicated over partitions) + slab sums
            bs = [outp.tile([128, 4 * H], F32, tag=f"bs{i}",
                            name=f"bs_{i}") for i in range(NQ)]
            ssum = [outp.tile([128, H], F32, tag=f"ss{i}",
                              name=f"ssum_{i}") for i in range(1, NQ)]
            corr = [outp.tile([128, 4 * H], F32, tag=f"corr{c}",
                              name=f"corr_{c}") for c in range(NQ)]
            of4s = []
            rcps = []
            sbns = []

            with (
                tc.tile_pool(name="psQ", bufs=1, space="PSUM") as psQ,
                tc.tile_pool(name="psE", bufs=1, space="PSUM") as psE,
            ):
                # ---- warmup: bf16 chain in the kv_acc bank ----
                wm = psQ.tile([128, 512], F32, tag="kvacc", bufs=1,
                              name="warm")
                for _ in range(N_WARM):
                    nc.tensor.matmul(wm, lhsT=ident_bf, rhs=scr,
                                     start=True, stop=True)

                for i in range(NQ):
                    # ---- projections for slab i ----
                    kv_acc = psQ.tile([128, 512], F32, tag="kvacc", bufs=1,
                                      name=f"kvacc_{i}")
                    q_acc = psQ.tile([128, 512], F32, tag="qacc", bufs=1,
                                     name=f"qacc_{i}")
                    # slab 0: x lands before y -> kv first; others: q first
                    if i == 0:
                        order = ([("kv", e) for e in range(ECH)]
                                 + [("q", e) for e in range(ECH)])
                    else:
                        order = ([("q", e) for e in range(ECH)]
                                 + [("kv", e) for e in range(ECH)])
                    for kind, e in order:
                        if kind == "kv":
                            nc.tensor.matmul(
                                kv_acc,
                                lhsT=w_kv[:, e * 128:(e + 1) * 128],
                                rhs=xTs[i][:, e * 512:(e + 1) * 512],
                                start=(e == 0), stop=(e == ECH - 1),
                            )
                        else:
                            nc.tensor.matmul(
                                q_acc,
                                lhsT=w_qq[:, e * 128:(e + 1) * 128],
                                rhs=yTs[i][:, e * 512:(e + 1) * 512],
                                start=(e == 0), stop=(e == ECH - 1),
                            )
                    sl = slice(i * 512, (i + 1) * 512)
                    nc.vector.tensor_scalar_add(
                        out=qTd[:, sl], in0=q_acc, scalar1=bqq)
                    nc.vector.tensor_scalar_add(
                        out=kTd[0:64, sl], in0=kv_acc[0:64, :],
                        scalar1=bkv[0:64, :])
                    nc.vector.tensor_scalar_add(
                        out=vT[:, sl], in0=kv_acc[64:128, :],
                        scalar1=bkv[64:128, :])
                    # kT dup on Pool SWDGE (off the input queues)
                    nc.gpsimd.dma_start(out=kTd[64:128, sl],
                                        in_=kTd[0:64, sl])
                    # v -> natural layout via Act-queue xbar transpose
                    nc.scalar.dma_start(
                        out=v_nat.rearrange(
                            "p (j h) -> p j h", h=H
                        )[:, 4 * i:4 * i + 4, :],
                        in_=vT[:, sl], transpose=True,
                    )
                    nc.gpsimd.tensor_copy(
                        v_aug.rearrange(
                            "p (j h) -> p j h", h=H + 1
                        )[:, 4 * i:4 * i + 4, 0:H],
                        v_nat.rearrange(
                            "p (j h) -> p j h", h=H
                        )[:, 4 * i:4 * i + 4, :],
                    )
                    # block sums of v, replicated across partitions
                    bsum_ps = psE.tile([128, 4 * H], F32, tag="pt", bufs=1,
                                       name=f"bsum_{i}")
                    nc.tensor.matmul(
                        bsum_ps, lhsT=ones_bf,
                        rhs=v_nat[:, i * 4 * H:(i + 1) * 4 * H],
                        start=True, stop=True)
                    nc.vector.tensor_copy(bs[i], bsum_ps)
                    if i >= 1:
                        bv4 = bs[i].rearrange("p (j h) -> p j h", h=H)
                        nc.vector.tensor_add(out=ssum[i - 1][:, :],
                                             in0=bv4[:, 0, :],
                                             in1=bv4[:, 1, :])
                        nc.vector.tensor_add(out=ssum[i - 1][:, :],
                                             in0=ssum[i - 1][:, :],
                                             in1=bv4[:, 2, :])
                        nc.vector.tensor_add(out=ssum[i - 1][:, :],
                                             in0=ssum[i - 1][:, :],
                                             in1=bv4[:, 3, :])

                    # ---- attention column i ----
                    c = i
                    pv = psE.tile([H + 1, 512], F32, tag="pv", bufs=1,
                                  name=f"pv_{c}")
                    nb = 4 * c + 4
                    for b2 in range(nb // 2):
                        st2 = psE.tile([128, 1024], F32, tag="st",
                                       bufs=2, name=f"st_{c}_{b2}")
                        ex2 = expp.tile([128, 1024], BF16, tag="expst",
                                        bufs=20, name=f"ex_{c}_{b2}")
                        diag = (2 * b2) // 4 == c
                        for half in range(2):
                            b = 2 * b2 + half
                            lo = 64 * half
                            o = half * 512
                            d0 = 128 * (b - 4 * c) if diag else 0
                            nc.tensor.matmul(
                                st2[:, o + d0:o + 512],
                                lhsT=kTd[lo:lo + 64,
                                         b * 128:(b + 1) * 128],
                                rhs=qTd[lo:lo + 64,
                                        c * 512 + d0:(c + 1) * 512],
                                start=True, stop=True,
                                tile_position=(lo, 0),
                            )
                        if diag:
                            for half in range(2):
                                b = 2 * b2 + half
                                d = b - 4 * c
                                o = half * 512
                                nc.scalar.activation(
                                    out=ex2[:, o + 128 * d:o + 512],
                                    in_=st2[:, o + 128 * d:o + 512],
                                    func=AF.Exp)
                                nc.gpsimd.affine_select(
                                    out=ex2[:, o + 128 * d:o + 128 * (d + 1)],
                                    in_=ex2[:, o + 128 * d:o + 128 * (d + 1)],
                                    pattern=[[1, 128]], channel_multiplier=-1,
                                    base=0,
                                    compare_op=mybir.AluOpType.is_ge,
                                    fill=1.0,
                                )
                        else:
                            nc.scalar.activation(out=ex2, in_=st2,
                                                 func=AF.Exp)
                        for half in range(2):
                            b = 2 * b2 + half
                            d0 = 128 * (b - 4 * c) if diag else 0
                            nc.tensor.matmul(
                                pv[:, d0:512],
                                lhsT=v_aug[:, b * (H + 1):(b + 1) * (H + 1)],
                                rhs=ex2[:, half * 512 + d0:(half + 1) * 512],
                                start=(b == 0),
                                stop=(b == nb - 1),
                            )
                    # ---- early finish: evac, transpose, den, rcp ----
                    sbn = outp.tile([H + 1, 512], BF16, tag="sbn", bufs=2,
                                    name=f"sbn_{c}")
                    nc.vector.tensor_copy(sbn, pv)
                    sbns.append(sbn)
                    pt4 = psE.tile([128, 4 * (H + 4)], BF16, tag="pt",
                                   bufs=1, name=f"pt4_{c}")
                    pt4v = pt4.rearrange("p (j h) -> p j h", h=H + 4)
                    for j4 in range(4):
                        nc.tensor.transpose(
                            pt4[:, j4 * (H + 4):j4 * (H + 4) + H + 1],
                            sbn[:, j4 * 128:(j4 + 1) * 128],
                            ident_bf[0:H + 1, 0:H + 1],
                        )
                    den4 = outp.tile([128, 4], F32, tag=f"den{c}",
                                     name=f"den4_{c}")
                    nc.vector.tensor_add(
                        out=den4,
                        in0=pt4v[:, :, H],
                        in1=denC[:, 4 * c:4 * c + 4])
                    rcp4 = outp.tile([128, 4], F32, tag=f"rcp{c}",
                                     name=f"rcp4_{c}")
                    nc.vector.reciprocal(rcp4, den4)
                    rcps.append(rcp4)
                    of4 = outp.tile([128, 4 * H], F32, tag=f"of{c}",
                                    name=f"of4_{c}")
                    nc.vector.tensor_copy(
                        of4.rearrange("p (j h) -> p j h", h=H),
                        pt4v[:, :, 0:H])
                    of4s.append(of4)

                # ---- tail: suffix corrections + normalize + store ----
                # corr[c][:, d*H:(d+1)*H] = sum_{blocks > 4c+d} blocksum
                # col 3: within-slab suffixes only (nothing after slab 3)
                bv3 = bs[3].rearrange("p (j h) -> p j h", h=H)
                nc.vector.memset(corr[3][:, 3 * H:4 * H], 0.0)
                nc.vector.tensor_copy(corr[3][:, 2 * H:3 * H], bv3[:, 3, :])
                nc.vector.tensor_add(out=corr[3][:, 1 * H:2 * H],
                                     in0=corr[3][:, 2 * H:3 * H],
                                     in1=bv3[:, 2, :])
                nc.vector.tensor_add(out=corr[3][:, 0:H],
                                     in0=corr[3][:, 1 * H:2 * H],
                                     in1=bv3[:, 1, :])
                # slab suffixes: suf(c) = sum_{s>c} slabsum(s)
                suf2 = ssum[2]                                 # slabsum(3)
                suf1 = outp.tile([128, H], F32, tag="suf1")
                nc.vector.tensor_add(out=suf1, in0=suf2, in1=ssum[1])
                suf0 = outp.tile([128, H], F32, tag="suf0")
                nc.vector.tensor_add(out=suf0, in0=suf1, in1=ssum[0])
                suf = [suf0, suf1, suf2]
                for c in range(3):
                    bvc = bs[c].rearrange("p (j h) -> p j h", h=H)
                    nc.vector.tensor_copy(corr[c][:, 3 * H:4 * H], suf[c])
                    nc.vector.tensor_add(out=corr[c][:, 2 * H:3 * H],
                                         in0=corr[c][:, 3 * H:4 * H],
                                         in1=bvc[:, 3, :])
                    nc.vector.tensor_add(out=corr[c][:, 1 * H:2 * H],
                                         in0=corr[c][:, 2 * H:3 * H],
                                         in1=bvc[:, 2, :])
                    nc.vector.tensor_add(out=corr[c][:, 0:H],
                                         in0=corr[c][:, 1 * H:2 * H],
                                         in1=bvc[:, 1, :])
                for c in range(NQ):
                    of4 = of4s[c]
                    nc.vector.tensor_add(out=of4, in0=of4, in1=corr[c])
                    of4v = of4.rearrange("p (j h) -> p j h", h=H)
                    for j4 in range(4):
                        nc.vector.tensor_scalar_mul(
                            out=of4v[:, j4, :], in0=of4v[:, j4, :],
                            scalar1=rcps[c][:, j4:j4 + 1])
                    nc.sync.dma_start(
                        out=out_ext[c * 512:(c + 1) * 512, :].rearrange(
                            "(j p) h -> p j h", p=128),
                        in_=of4v)

    _split_multi_waits(nc)
    return nc


LAST_EXEC_TIME_NS = None
_CACHE = {}


def kernel(x, y, Wq, bq, Wk, bk, Wv, bv):
    """Full-input entry point: shards batch over 8 NeuronCores (one batch
    element per core), runs the Bass kernel, gathers the full output."""
    global LAST_EXEC_TIME_NS
    import os

    import ml_dtypes
    from concourse.bass_utils import run_bass_kernel_spmd

    if "nc" not in _CACHE:
        _CACHE["nc"] = _build()
    nc = _CACHE["nc"]

    bf = ml_dtypes.bfloat16
    x = np.asarray(x, np.float32)
    y = np.asarray(y, np.float32)

    # weight packing: [128, ECH, 128] -> [128, ECH*128]
    wk8 = (np.asarray(Wk, np.float32) * 0.125).astype(bf).reshape(ECH, 128, H)
    wv2 = np.asarray(Wv, np.float32).astype(bf).reshape(ECH, 128, H)
    wkv = np.ascontiguousarray(
        np.concatenate([wk8, wv2], axis=2).transpose(1, 0, 2)
    ).reshape(128, ECH * 128)
    wq2 = np.asarray(Wq, np.float32).astype(bf).reshape(ECH, 128, H)
    wqq = np.ascontiguousarray(
        np.concatenate([wq2, wq2], axis=2).transpose(1, 0, 2)
    ).reshape(128, ECH * 128)
    bq1 = np.asarray(bq, np.float32).reshape(H, 1)
    bqqc = np.ascontiguousarray(np.concatenate([bq1, bq1], axis=0))
    bkvc = np.ascontiguousarray(np.concatenate(
        [np.asarray(bk, np.float32).reshape(H, 1) * 0.125,
         np.asarray(bv, np.float32).reshape(H, 1)], axis=0))

    def slabs(a):
        # [S, E] f32 -> bf16 [E, S] -> [NQ, 128, ECH*512] slab-major
        t = a.astype(bf).T.reshape(ECH, 128, NQ, 512)
        return np.ascontiguousarray(
            t.transpose(2, 1, 0, 3).reshape(NQ, 128, ECH * 512))

    in_maps = []
    for b in range(8):
        in_maps.append({
            "x": slabs(x[b]), "y": slabs(y[b]),
            "wkv": wkv, "wqq": wqq,
            "bqq": bqqc, "bkv": bkvc,
        })

    trace = bool(os.environ.get("ATTN_TRACE"))
    res = run_bass_kernel_spmd(nc, in_maps, core_ids=list(range(8)),
                               trace=trace)
    if trace:
        LAST_EXEC_TIME_NS = res.exec_time_ns
    return np.stack([res.results[i]["out"] for i in range(8)]).astype(
        np.float32)


# revision 10
# speedup vs baseline: 1.1693x; 1.1391x over previous
"""Trainium2 Bass kernel for nn_AttentionHead (single-head attention with
pre-softmax tril zeroing). B=8, S=2048, E=1024, H=64.

Sharding: data-parallel over batch - one batch element per NeuronCore.

v11 design (from v10 trace analysis):
  - input DMAs all on the sync HWDGE queue, emitted first, in order
    wkv, x0, wqq, y0, y1, x1, y2, y3, x2, x3; kT dups moved to Pool
    SWDGE and v-transposes to the Act queue so the input stream never
    serializes behind compute-dependent DMAs (v10's 14us DMA hole)
  - bf16 warmup matmul chain keeps the PE busy (and its p-state ramped)
    through the ~11us DMA/program preamble; it shares the kv_acc PSUM
    bank so slab 0's first matmul chains with zero gap
  - q projection uses [Wq|Wq] (M=128): partitions 64:128 of qTd are
    produced by the matmul itself - no SBUF-SBUF dup DMA
  - diagonal pv blocks are d0-trimmed like the scores; the skipped
    all-ones columns are replaced by closed-form per-128-step
    corrections applied POST-transpose: denominator constants are
    compile-time memsets, v-suffix sums come from ones[128,128] @ v_nat
    matmuls (partition-replicated for free). Each column's transpose/
    reciprocal runs right after its pv; only a [128,256] add + 4 muls +
    store remain for the tail -> tail shrinks from ~12us to ~2us
"""

import numpy as np

import concourse.bass as bass
import concourse.mybir as mybir
from concourse.tile import TileContext

S, E, H = 2048, 1024, 64
SC = S // 128   # 16 key blocks
ECH = E // 128  # 8 e-chunks
NQ = 4          # q-chunks (slabs) of 512
F32 = mybir.dt.float32
BF16 = mybir.dt.bfloat16
AF = mybir.ActivationFunctionType
N_WARM = 26     # bf16 warmup matmuls covering the DMA/program preamble

_SPLIT_COUNTER = [0]


def _split_multi_waits(nc, ev_cap=1):
    """This container's walrus build accepts at most 1 sem-wait per
    instruction (2 on EventSemaphore); move excess waits onto EvSem
    instructions inserted just before, on the same engine."""
    for f in nc.m.functions:
        for bb in f.blocks:
            ins_list = bb.instructions
            need = False
            for ins in ins_list:
                si = ins.sync_info
                if si is None:
                    continue
                cap = 2 if isinstance(ins, mybir.InstEventSemaphore) else 1
                if len(si.on_wait) > cap:
                    need = True
                    break
            if not need:
                continue
            new_list = []
            for ins in ins_list:
                si = ins.sync_info
                cap = 2 if isinstance(ins, mybir.InstEventSemaphore) else 1
                if si is not None and len(si.on_wait) > cap:
                    waits = list(si.on_wait)
                    keep = waits[-cap:]
                    head = waits[:-cap]
                    for i in range(0, len(head), ev_cap):
                        _SPLIT_COUNTER[0] += 1
                        ev = mybir.InstEventSemaphore(
                            name=f"EVSPLIT-{_SPLIT_COUNTER[0]}",
                            engine=ins.engine,
                            ins=[],
                            outs=[],
                            sync_info=mybir.SyncInfo(
                                on_wait=head[i:i + ev_cap], on_update=[]
                            ),
                        )
                        nc.register_instruction(ev)
                        new_list.append(ev)
                    ins.sync_info = mybir.SyncInfo(
                        on_wait=keep, on_update=list(si.on_update)
                    )
                new_list.append(ins)
            bb.instructions = new_list


def _build():
    nc = bass.Bass()
    x_ext = nc.declare_dram_parameter("x", [NQ, 128, ECH * 512], BF16,
                                      isOutput=False)
    y_ext = nc.declare_dram_parameter("y", [NQ, 128, ECH * 512], BF16,
                                      isOutput=False)
    wkv_ext = nc.declare_dram_parameter("wkv", [128, ECH * 128], BF16,
                                        isOutput=False)
    wqq_ext = nc.declare_dram_parameter("wqq", [128, ECH * 128], BF16,
                                        isOutput=False)
    bqq_ext = nc.declare_dram_parameter("bqq", [128, 1], F32, isOutput=False)
    bkv_ext = nc.declare_dram_parameter("bkv", [128, 1], F32, isOutput=False)
    out_ext = nc.declare_dram_parameter("out", [S, H], F32, isOutput=True)

    with TileContext(nc) as tc:
        with (
            tc.tile_pool(name="consts", bufs=1) as consts,
            tc.tile_pool(name="bigT", bufs=1) as bigT,
            tc.tile_pool(name="expp", bufs=3) as expp,
            tc.tile_pool(name="outp", bufs=2) as outp,
        ):
            # ---- warmup source first: warmup depends ONLY on this ----
            scr = consts.tile([128, 512], BF16, tag="scr")
            nc.vector.memset(scr, 0.0)
            # ---- constants ----
            ident_bf = consts.tile([128, 128], BF16)
            nc.vector.memset(ident_bf, 1.0)
            nc.gpsimd.affine_select(
                out=ident_bf, in_=ident_bf,
                pattern=[[-1, 128]], channel_multiplier=1, base=0,
                compare_op=mybir.AluOpType.is_equal, fill=0.0,
            )
            ones_bf = consts.tile([128, 128], BF16, tag="ones")
            nc.gpsimd.memset(ones_bf, 1.0)
            # denominator constants per (col c, step d): S-128-512c-128d
            denC = consts.tile([128, 16], F32, tag="denC")
            for c in range(NQ):
                for dstep in range(4):
                    nc.vector.memset(
                        denC[:, 4 * c + dstep:4 * c + dstep + 1],
                        float(S - 128 - 512 * c - 128 * dstep))

            # ---- biases on SWDGE ----
            bqq = consts.tile([128, 1], F32, tag="bqq", name="bias_qq")
            nc.gpsimd.dma_start(out=bqq, in_=bqq_ext[:, :])
            bkv = consts.tile([128, 1], F32, tag="bkv", name="bias_kv")
            nc.gpsimd.dma_start(out=bkv, in_=bkv_ext[:, :])

            # ---- weights + input slabs: sync queue, load order matters ----
            w_kv = consts.tile([128, ECH * 128], BF16, tag="w_kv")
            w_qq = consts.tile([128, ECH * 128], BF16, tag="w_qq")
            xTs = [bigT.tile([128, ECH * 512], BF16, tag=f"xT{i}",
                             name=f"xT_{i}") for i in range(NQ)]
            yTs = [bigT.tile([128, ECH * 512], BF16, tag=f"yT{i}",
                             name=f"yT_{i}") for i in range(NQ)]
            nc.sync.dma_start(out=w_kv, in_=wkv_ext[:, :])
            nc.sync.dma_start(out=xTs[0], in_=x_ext[0])
            nc.sync.dma_start(out=w_qq, in_=wqq_ext[:, :])
            nc.sync.dma_start(out=yTs[0], in_=y_ext[0])
            nc.sync.dma_start(out=yTs[1], in_=y_ext[1])
            nc.sync.dma_start(out=xTs[1], in_=x_ext[1])
            nc.sync.dma_start(out=yTs[2], in_=y_ext[2])
            nc.sync.dma_start(out=yTs[3], in_=y_ext[3])
            nc.sync.dma_start(out=xTs[2], in_=x_ext[2])
            nc.sync.dma_start(out=xTs[3], in_=x_ext[3])

            # qTd: both halves from [Wq|Wq]; kTd: rows 0:64 evac, 64:128 dup
            qTd = bigT.tile([128, S], BF16, tag="qTd")
            kTd = bigT.tile([128, S], BF16, tag="kTd")
            vT = bigT.tile([64, S], BF16, tag="vT")
            v_aug = bigT.tile([128, SC * (H + 1)], BF16, tag="vaug")
            nc.gpsimd.memset(v_aug, 1.0)
            # per-slab block-sum (replicated over partitions) + slab sums
            bs = [outp.tile([128, 4 * H], F32, tag=f"bs{i}",
                            name=f"bs_{i}") for i in range(NQ)]
            ssum = [outp.tile([128, H], F32, tag=f"ss{i}",
                              name=f"ssum_{i}") for i in range(1, NQ)]
            corr = [outp.tile([128, 4 * H], F32, tag=f"corr{c}",
                              name=f"corr_{c}") for c in range(NQ)]
            of4s = []
            rcps = []
            sbns = []

            with (
                tc.tile_pool(name="psQ", bufs=1, space="PSUM") as psQ,
                tc.tile_pool(name="psE", bufs=1, space="PSUM") as psE,
            ):
                # ---- warmup: bf16 chain in the kv_acc bank ----
                wm = psQ.tile([128, 512], F32, tag="kvacc", bufs=1,
                              name="warm")
                for _ in range(N_WARM):
                    nc.tensor.matmul(wm, lhsT=scr[:, 0:128], rhs=scr,
                                     start=True, stop=True)

                for i in range(NQ):
                    # ---- projections for slab i ----
                    kv_acc = psQ.tile([128, 512], F32, tag="kvacc", bufs=1,
                                      name=f"kvacc_{i}")
                    q_acc = psQ.tile([128, 512], F32, tag="qacc", bufs=1,
                                     name=f"qacc_{i}")
                    # slab 0: x lands before y -> kv first; others: q first
                    if i == 0:
                        order = ([("kv", e) for e in range(ECH)]
                                 + [("q", e) for e in range(ECH)])
                    else:
                        order = ([("q", e) for e in range(ECH)]
                                 + [("kv", e) for e in range(ECH)])
                    for kind, e in order:
                        if kind == "kv":
                            nc.tensor.matmul(
                                kv_acc,
                                lhsT=w_kv[:, e * 128:(e + 1) * 128],
                                rhs=xTs[i][:, e * 512:(e + 1) * 512],
                                start=(e == 0), stop=(e == ECH - 1),
                            )
                        else:
                            nc.tensor.matmul(
                                q_acc,
                                lhsT=w_qq[:, e * 128:(e + 1) * 128],
                                rhs=yTs[i][:, e * 512:(e + 1) * 512],
                                start=(e == 0), stop=(e == ECH - 1),
                            )
                    sl = slice(i * 512, (i + 1) * 512)
                    nc.vector.tensor_scalar_add(
                        out=qTd[:, sl], in0=q_acc, scalar1=bqq)
                    nc.vector.tensor_scalar_add(
                        out=kTd[0:64, sl], in0=kv_acc[0:64, :],
                        scalar1=bkv[0:64, :])
                    nc.vector.tensor_scalar_add(
                        out=vT[:, sl], in0=kv_acc[64:128, :],
                        scalar1=bkv[64:128, :])
                    # kT dup on Pool SWDGE (off the input queues)
                    nc.gpsimd.dma_start(out=kTd[64:128, sl],
                                        in_=kTd[0:64, sl])
                    # v -> natural layout via PE transposes (keeps ALL
                    # compute-dependent traffic off the HWDGE queues, whose
                    # lane chain otherwise serializes the input stream)
                    vav = v_aug.rearrange("p (j h) -> p j h", h=H + 1)
                    for blk in range(4):
                        vt_ps = (psE.tile([128, H], BF16, tag="pt", bufs=1,
                                          name=f"vt_{i}_{blk}")
                                 if blk % 2 == 0 else
                                 psQ.tile([128, H], BF16, tag="kvacc",
                                          bufs=1, name=f"vt_{i}_{blk}"))
                        nc.tensor.transpose(
                            vt_ps,
                            vT[:, i * 512 + blk * 128:
                               i * 512 + (blk + 1) * 128],
                            ident_bf[0:64, 0:64],
                        )
                        nc.vector.tensor_copy(
                            vav[:, 4 * i + blk, 0:H], vt_ps)
                    # block sums of v, replicated across partitions
                    bsum_ps = psE.tile([128, 4 * H], F32, tag="pt", bufs=1,
                                       name=f"bsum_{i}")
                    for blk in range(4):
                        nc.tensor.matmul(
                            bsum_ps[:, blk * H:(blk + 1) * H],
                            lhsT=ones_bf,
                            rhs=vav[:, 4 * i + blk, 0:H],
                            start=True, stop=True)
                    nc.vector.tensor_copy(bs[i], bsum_ps)
                    if i >= 1:
                        bv4 = bs[i].rearrange("p (j h) -> p j h", h=H)
                        nc.vector.tensor_add(out=ssum[i - 1][:, :],
                                             in0=bv4[:, 0, :],
                                             in1=bv4[:, 1, :])
                        nc.vector.tensor_add(out=ssum[i - 1][:, :],
                                             in0=ssum[i - 1][:, :],
                                             in1=bv4[:, 2, :])
                        nc.vector.tensor_add(out=ssum[i - 1][:, :],
                                             in0=ssum[i - 1][:, :],
                                             in1=bv4[:, 3, :])

                    # ---- attention column i ----
                    c = i
                    pv = psE.tile([H + 1, 512], F32, tag="pv", bufs=1,
                                  name=f"pv_{c}")
                    nb = 4 * c + 4
                    for b2 in range(nb // 2):
                        st2 = psE.tile([128, 1024], F32, tag="st",
                                       bufs=2, name=f"st_{c}_{b2}")
                        ex2 = expp.tile([128, 1024], BF16, tag="expst",
                                        bufs=20, name=f"ex_{c}_{b2}")
                        diag = (2 * b2) // 4 == c
                        for half in range(2):
                            b = 2 * b2 + half
                            lo = 64 * half
                            o = half * 512
                            d0 = 128 * (b - 4 * c) if diag else 0
                            nc.tensor.matmul(
                                st2[:, o + d0:o + 512],
                                lhsT=kTd[lo:lo + 64,
                                         b * 128:(b + 1) * 128],
                                rhs=qTd[lo:lo + 64,
                                        c * 512 + d0:(c + 1) * 512],
                                start=True, stop=True,
                                tile_position=(lo, 0),
                            )
                        if diag:
                            for half in range(2):
                                b = 2 * b2 + half
                                d = b - 4 * c
                                o = half * 512
                                nc.scalar.activation(
                                    out=ex2[:, o + 128 * d:o + 512],
                                    in_=st2[:, o + 128 * d:o + 512],
                                    func=AF.Exp)
                                nc.gpsimd.affine_select(
                                    out=ex2[:, o + 128 * d:o + 128 * (d + 1)],
                                    in_=ex2[:, o + 128 * d:o + 128 * (d + 1)],
                                    pattern=[[1, 128]], channel_multiplier=-1,
                                    base=0,
                                    compare_op=mybir.AluOpType.is_ge,
                                    fill=1.0,
                                )
                        else:
                            nc.scalar.activation(out=ex2, in_=st2,
                                                 func=AF.Exp)
                        for half in range(2):
                            b = 2 * b2 + half
                            d0 = 128 * (b - 4 * c) if diag else 0
                            nc.tensor.matmul(
                                pv[:, d0:512],
                                lhsT=v_aug[:, b * (H + 1):(b + 1) * (H + 1)],
                                rhs=ex2[:, half * 512 + d0:(half + 1) * 512],
                                start=(b == 0),
                                stop=(b == nb - 1),
                            )
                    # ---- early finish: evac, transpose, den, rcp ----
                    sbn = outp.tile([H + 1, 512], BF16, tag="sbn", bufs=2,
                                    name=f"sbn_{c}")
                    nc.vector.tensor_copy(sbn, pv)
                    sbns.append(sbn)
                    pt4 = psE.tile([128, 4 * (H + 4)], BF16, tag="pt",
                                   bufs=1, name=f"pt4_{c}")
                    pt4v = pt4.rearrange("p (j h) -> p j h", h=H + 4)
                    for j4 in range(4):
                        nc.tensor.transpose(
                            pt4[:, j4 * (H + 4):j4 * (H + 4) + H + 1],
                            sbn[:, j4 * 128:(j4 + 1) * 128],
                            ident_bf[0:H + 1, 0:H + 1],
                        )
                    den4 = outp.tile([128, 4], F32, tag=f"den{c}",
                                     name=f"den4_{c}")
                    nc.vector.tensor_add(
                        out=den4,
                        in0=pt4v[:, :, H],
                        in1=denC[:, 4 * c:4 * c + 4])
                    rcp4 = outp.tile([128, 4], F32, tag=f"rcp{c}",
                                     name=f"rcp4_{c}")
                    nc.vector.reciprocal(rcp4, den4)
                    rcps.append(rcp4)
                    of4 = outp.tile([128, 4 * H], F32, tag=f"of{c}",
                                    name=f"of4_{c}")
                    nc.vector.tensor_copy(
                        of4.rearrange("p (j h) -> p j h", h=H),
                        pt4v[:, :, 0:H])
                    of4s.append(of4)

                # ---- tail: suffix corrections + normalize + store ----
                # corr[c][:, d*H:(d+1)*H] = sum_{blocks > 4c+d} blocksum
                # col 3: within-slab suffixes only (nothing after slab 3)
                bv3 = bs[3].rearrange("p (j h) -> p j h", h=H)
                nc.vector.memset(corr[3][:, 3 * H:4 * H], 0.0)
                nc.vector.tensor_copy(corr[3][:, 2 * H:3 * H], bv3[:, 3, :])
                nc.vector.tensor_add(out=corr[3][:, 1 * H:2 * H],
                                     in0=corr[3][:, 2 * H:3 * H],
                                     in1=bv3[:, 2, :])
                nc.vector.tensor_add(out=corr[3][:, 0:H],
                                     in0=corr[3][:, 1 * H:2 * H],
                                     in1=bv3[:, 1, :])
                # slab suffixes: suf(c) = sum_{s>c} slabsum(s)
                suf2 = ssum[2]                                 # slabsum(3)
                suf1 = outp.tile([128, H], F32, tag="suf1")
                nc.vector.tensor_add(out=suf1, in0=suf2, in1=ssum[1])
                suf0 = outp.tile([128, H], F32, tag="suf0")
                nc.vector.tensor_add(out=suf0, in0=suf1, in1=ssum[0])
                suf = [suf0, suf1, suf2]
                for c in range(3):
                    bvc = bs[c].rearrange("p (j h) -> p j h", h=H)
                    nc.vector.tensor_copy(corr[c][:, 3 * H:4 * H], suf[c])
                    nc.vector.tensor_add(out=corr[c][:, 2 * H:3 * H],
                                         in0=corr[c][:, 3 * H:4 * H],
                                         in1=bvc[:, 3, :])
                    nc.vector.tensor_add(out=corr[c][:, 1 * H:2 * H],
                                         in0=corr[c][:, 2 * H:3 * H],
                                         in1=bvc[:, 2, :])
                    nc.vector.tensor_add(out=corr[c][:, 0:H],
                                         in0=corr[c][:, 1 * H:2 * H],
                                         in1=bvc[:, 1, :])
                for c in range(NQ):
                    of4 = of4s[c]
                    nc.vector.tensor_add(out=of4, in0=of4, in1=corr[c])
                    of4v = of4.rearrange("p (j h) -> p j h", h=H)
                    for j4 in range(4):
                        nc.vector.tensor_scalar_mul(
                            out=of4v[:, j4, :], in0=of4v[:, j4, :],
                            scalar1=rcps[c][:, j4:j4 + 1])
                    nc.sync.dma_start(
                        out=out_ext[c * 512:(c + 1) * 512, :].rearrange(
                            "(j p) h -> p j h", p=128),
                        in_=of4v)

    _split_multi_waits(nc)
    return nc


LAST_EXEC_TIME_NS = None
_CACHE = {}


def kernel(x, y, Wq, bq, Wk, bk, Wv, bv):
    """Full-input entry point: shards batch over 8 NeuronCores (one batch
    element per core), runs the Bass kernel, gathers the full output."""
    global LAST_EXEC_TIME_NS
    import os

    import ml_dtypes
    from concourse.bass_utils import run_bass_kernel_spmd

    if "nc" not in _CACHE:
        _CACHE["nc"] = _build()
    nc = _CACHE["nc"]

    bf = ml_dtypes.bfloat16
    x = np.asarray(x, np.float32)
    y = np.asarray(y, np.float32)

    # weight packing: [128, ECH, 128] -> [128, ECH*128]
    wk8 = (np.asarray(Wk, np.float32) * 0.125).astype(bf).reshape(ECH, 128, H)
    wv2 = np.asarray(Wv, np.float32).astype(bf).reshape(ECH, 128, H)
    wkv = np.ascontiguousarray(
        np.concatenate([wk8, wv2], axis=2).transpose(1, 0, 2)
    ).reshape(128, ECH * 128)
    wq2 = np.asarray(Wq, np.float32).astype(bf).reshape(ECH, 128, H)
    wqq = np.ascontiguousarray(
        np.concatenate([wq2, wq2], axis=2).transpose(1, 0, 2)
    ).reshape(128, ECH * 128)
    bq1 = np.asarray(bq, np.float32).reshape(H, 1)
    bqqc = np.ascontiguousarray(np.concatenate([bq1, bq1], axis=0))
    bkvc = np.ascontiguousarray(np.concatenate(
        [np.asarray(bk, np.float32).reshape(H, 1) * 0.125,
         np.asarray(bv, np.float32).reshape(H, 1)], axis=0))

    def slabs(a):
        # [S, E] f32 -> bf16 [E, S] -> [NQ, 128, ECH*512] slab-major
        t = a.astype(bf).T.reshape(ECH, 128, NQ, 512)
        return np.ascontiguousarray(
            t.transpose(2, 1, 0, 3).reshape(NQ, 128, ECH * 512))

    in_maps = []
    for b in range(8):
        in_maps.append({
            "x": slabs(x[b]), "y": slabs(y[b]),
            "wkv": wkv, "wqq": wqq,
            "bqq": bqqc, "bkv": bkvc,
        })

    trace = bool(os.environ.get("ATTN_TRACE"))
    res = run_bass_kernel_spmd(nc, in_maps, core_ids=list(range(8)),
                               trace=trace)
    if trace:
        LAST_EXEC_TIME_NS = res.exec_time_ns
    return np.stack([res.results[i]["out"] for i in range(8)]).astype(
        np.float32)
